# revision 16
# baseline (speedup 1.0000x reference)
"""Per-pixel dynamic-filter 5x5 convolution (KPN-style) on 8 TRN2 NeuronCores.

Math: out[b,h,w] = sum_{di,dj,c} img[b, h+di-2, w+dj-2, c] * filts[b, h, w, (di*5+dj)*3+c]
Shapes: img [4,512,512,3] f32, filts [4,512,512,75] f32 -> out [4,512,512] f32.

Strategy (pure data parallel, no cross-core comms):
  - 8 shards = (batch b) x (H half); each core owns a [256, 512] output slab,
    processed as 2 h-tiles of 128 rows (ht outer loop, output overlapped).
  - filts int8-quantized on host (scale S, ~9e-3 rel err vs 2e-2 budget) to
    halve the dominant DMA stream. Per (ht,di) the 15 (dj,c) planes are
    divided across engines to balance the machine:
      dj in {0,2}   (6 planes): ACT dequant -> DVE fp16 2x TT
      dj = 1        (3 planes): ACT dequant -> DVE fp16 2x TT
      dj = 3        (3 planes): DVE scalar_tensor_tensor on raw int8 (fused)
      dj = 4        (3 planes): GPSIMD int8 x fp16 TT (unscaled products;
        scale folded into an S*I stationary at the PE accumulate step)
  - img fp16; one replicated-row DMA per h-tile carries all 5 di row-shifted
    copies so engines never need partition shifts (DMA count stays tiny).
  - odd dj operands are 4B-aligned by host-shifting those filts by +1 in w
    (x-offset dj+1, psum target cols [1:512)); the single real missing w=0
    term (dj=3) is restored via a small TT add tree merged at eviction.
  - TensorE accumulates the planes into one fp32 PSUM bank per ht via
    identity (or S*identity) matmuls; ACT evicts cols [1:512) to fp16, DVE
    merges col 0 with the edge sum; DMA out.
  - Dummy-matmul warmup in the first DMA shadow lifts the PE HAM throttle.
"""

import sys

sys.path.insert(0, "/opt/trn_rl_repo")

import numpy as np

from concourse import bass, bacc, mybir
from concourse.tile import TileContext
from concourse.bass_utils import run_bass_kernel_spmd

B, H, W, C = 4, 512, 512, 3
K = 5
N_CORES = 8
HSH = H // 2  # 256 rows per shard
N_HT = HSH // 128  # 2 h-tiles per shard
XP = W + 6  # padded x extent: x = w+2, w in [-2, 516)
NPL = K * C  # 15 planes per (ht, di)
QSCALE = float(np.float16(4.5 / 127.0))  # int8 quant scale, fp16-exact
N_WARMUP_MM = 12

_F16 = mybir.dt.float16
_F32 = mybir.dt.float32
_I8 = mybir.dt.int8

_NC = None

# plane order in DRAM: [dj0 c0-2][dj2 c0-2][dj1 c0-2][dj3 c0-2][dj4 c0-2]
# index ranges:         0:3        3:6        6:9       9:12      12:15


def _win(img_t, di, ngrp, x0, stride):
    """img operand AP [p][grp:ngrp step stride][c:3][x:512] at x offset x0."""
    base = img_t[:, di, :, x0:] if x0 else img_t[:, di]
    ap = base.unsqueeze(1).broadcast_to((128, ngrp, C, XP - x0)).copy()
    ap.ap[1] = [stride, ngrp]
    ap.ap[3] = [1, W]
    return ap


def build_nc():
    """Build the single-core Bass program (identical on all 8 cores)."""
    nc = bacc.Bacc("TRN2")
    img_d = nc.declare_dram_parameter("img", [HSH + 4, C, XP], _F16, isOutput=False)
    filts_d = nc.declare_dram_parameter(
        "filts", [128, N_HT, K, NPL, W], _I8, isOutput=False
    )
    edge_d = nc.declare_dram_parameter("edge", [128, N_HT, K, C], _F16, isOutput=False)
    ident_d = nc.declare_dram_parameter("ident", [128, 128], _F16, isOutput=False)
    out_d = nc.declare_dram_parameter("out", [N_HT, 128, W], _F16, isOutput=True)

    with TileContext(nc) as tc:
        with (
            tc.tile_pool(name="const", bufs=1) as constp,
            tc.tile_pool(name="imgp", bufs=2) as imgp,
            tc.tile_pool(name="qp", bufs=4) as qp,
            tc.tile_pool(name="fp", bufs=4) as fp,
            tc.tile_pool(name="prodp", bufs=4) as prodp,
            tc.tile_pool(name="accp", bufs=2) as accp,
            tc.tile_pool(name="outp", bufs=2) as outp,
            tc.tile_pool(name="psump", bufs=2, space="PSUM") as psump,
            tc.tile_pool(name="wpsump", bufs=1, space="PSUM") as wpsump,
        ):
            id_t = constp.tile([128, 128], _F16)
            nc.sync.dma_start(out=id_t[:], in_=ident_d[:])
            ids_t = constp.tile([128, 128], _F16, tag="ids")
            nc.scalar.mul(ids_t[:], id_t[:], QSCALE)  # S*I for unscaled planes
            edge_t = constp.tile([128, N_HT, K, C], _F16, tag="edge")
            nc.sync.dma_start(out=edge_t[:], in_=edge_d[:])

            # PE warmup: dummy matmuls in the first DMAs' shadow lift HAM.
            wsrc = constp.tile([128, 512], _F16, tag="wsrc")
            nc.vector.memset(wsrc[:], 0.0)
            wps = wpsump.tile([128, 512], _F32)
            for _ in range(N_WARMUP_MM):
                nc.tensor.matmul(wps[:], wsrc[:, :128], wsrc[:], start=True, stop=True)

            for ht in range(N_HT):
                # One DMA (split in two for startup) for all 5 di row-shifted
                # img copies: tile[p, di, c, x] = img_d[ht*128 + p + di, c, x]
                img_t = imgp.tile([128, K, C, XP], _F16, tag="img", name=f"img{ht}")
                nc.sync.dma_start(
                    out=img_t[:, 0], in_=img_d[ht * 128 : ht * 128 + 128, :, :]
                )
                src = img_d[ht * 128 + 1 : ht * 128 + 129, :, :]
                src = src.unsqueeze(1).broadcast_to((128, K - 1, C, XP)).copy()
                src.ap[1] = [C * XP, K - 1]  # di steps one whole row
                nc.sync.dma_start(out=img_t[:, 1:], in_=src)

                ps = psump.tile([128, W], _F32, tag="ps", name=f"ps{ht}")

                for di in range(K):
                    q_t = qp.tile([128, NPL, W], _I8, tag="q", name=f"q{ht}{di}")
                    nc.sync.dma_start(out=q_t[:], in_=filts_d[:, ht, di])

                    # ACT dequants planes 0:9 (dj 0,2,1)
                    f_t = fp.tile([128, 9, W], _F16, tag="f", name=f"f{ht}{di}")
                    nc.scalar.mul(f_t[:], q_t[:, :9], QSCALE)

                    # DVE TT-A: dj in {0,2}, x0 {0,2}  (6 planes, fp16 2x)
                    p_a = prodp.tile([128, 6, W], _F16, tag="pa", name=f"pa{ht}{di}")
                    nc.vector.tensor_tensor(
                        p_a[:].rearrange("p (a c) x -> p a c x", a=2),
                        _win(img_t, di, 2, 0, 2),
                        f_t[:, :6].rearrange("p (a c) x -> p a c x", a=2),
                        mybir.AluOpType.mult,
                    )
                    # DVE TT-B: dj=1, x0=2  (3 planes, fp16 2x)
                    p_b = prodp.tile([128, 3, W], _F16, tag="pb", name=f"pb{ht}{di}")
                    nc.vector.tensor_tensor(
                        p_b[:].rearrange("p (a c) x -> p a c x", a=1),
                        _win(img_t, di, 1, 2, 2),
                        f_t[:, 6:9].rearrange("p (a c) x -> p a c x", a=1),
                        mybir.AluOpType.mult,
                    )
                    # DVE STT: dj=3, x0=4 (3 planes, int8 fused dequant)
                    p_s = prodp.tile([128, 3, W], _F16, tag="ps2", name=f"pS{ht}{di}")
                    nc.vector.scalar_tensor_tensor(
                        p_s[:],
                        q_t[:, 9:12],
                        QSCALE,
                        img_t[:, di, :, 4 : 4 + W],
                        mybir.AluOpType.mult,
                        mybir.AluOpType.mult,
                    )
                    # GPSIMD TT: dj=4 (3 planes), int8 unscaled; their
                    # matmuls use the S*I stationary.
                    p_g3 = prodp.tile([128, 3, W], _F16, tag="pg3", name=f"pg3{ht}{di}")
                    nc.gpsimd.tensor_tensor(
                        p_g3[:],
                        q_t[:, 12:15],
                        img_t[:, di, :, 4 : 4 + W],
                        mybir.AluOpType.mult,
                    )

                    # accumulate 15 planes into this ht's psum bank.
                    # evens (full 512): p_a (id), p_g3 (S*I)
                    for k in range(6):
                        nc.tensor.matmul(
                            ps[:],
                            id_t[:],
                            p_a[:, k, :],
                            start=(di == 0 and k == 0),
                            stop=False,
                        )
                    for k in range(3):
                        nc.tensor.matmul(
                            ps[:], ids_t[:], p_g3[:, k, :], start=False, stop=False
                        )
                    # odds (cols [1:512)): p_b (id), p_s (id), p_g1 (S*I)
                    for k in range(3):
                        nc.tensor.matmul(
                            ps[:, 1:W],
                            id_t[:],
                            p_b[:, k, 0 : W - 1],
                            start=False,
                            stop=False,
                        )
                    for k in range(3):
                        nc.tensor.matmul(
                            ps[:, 1:W],
                            id_t[:],
                            p_s[:, k, 0 : W - 1],
                            start=False,
                            stop=(di == K - 1 and k == 2),
                        )

                # w=0 edge terms (dj=3 only): sum_{di,c} img[x=3] * edge filts
                ep = accp.tile([128, K, C], _F32, tag="ep", name=f"ep{ht}")
                nc.vector.tensor_tensor(
                    ep[:], img_t[:, :, :, 3], edge_t[:, ht], mybir.AluOpType.mult
                )
                ec = accp.tile([128, K], _F32, tag="ec", name=f"ec{ht}")
                nc.vector.tensor_tensor(
                    ec[:], ep[:, :, 0], ep[:, :, 1], mybir.AluOpType.add
                )
                nc.vector.tensor_tensor(
                    ec[:], ec[:], ep[:, :, 2], mybir.AluOpType.add
                )
                e2 = accp.tile([128, 2], _F32, tag="e2", name=f"e2{ht}")
                nc.vector.tensor_tensor(
                    e2[:], ec[:, 0:2], ec[:, 2:4], mybir.AluOpType.add
                )
                e1 = accp.tile([128, 1], _F32, tag="e1", name=f"e1{ht}")
                nc.vector.tensor_tensor(
                    e1[:], e2[:, 0:1], e2[:, 1:2], mybir.AluOpType.add
                )
                nc.vector.tensor_tensor(
                    e1[:], e1[:], ec[:, 4:5], mybir.AluOpType.add
                )

                # evict: ACT copies cols [1:512); DVE merges edge into col 0
                o_t = outp.tile([128, W], _F16, tag="ot", name=f"ot{ht}")
                nc.scalar.copy(out=o_t[:, 1:], in_=ps[:, 1:])
                nc.vector.tensor_tensor(
                    o_t[:, 0:1], ps[:, 0:1], e1[:], mybir.AluOpType.add
                )
                nc.sync.dma_start(out=out_d[ht], in_=o_t[:])

    nc.compile()
    return nc


def get_nc():
    global _NC
    if _NC is None:
        _NC = build_nc()
    return _NC


def prepare_in_maps(img_stack: np.ndarray, filts: np.ndarray):
    """Shard + reformat FULL fp32 inputs into per-core input maps."""
    ident = np.eye(128, dtype=np.float16)
    in_maps = []
    for core in range(N_CORES):
        b, hh = divmod(core, 2)
        h0 = hh * HSH
        # img: pad h by 2 each side, w by 2 left / 4 right -> [516, 518, 3]
        padded = np.pad(img_stack[b], ((2, 2), (2, XP - W - 2), (0, 0)))
        shard = padded[h0 : h0 + HSH + 4]  # rows h0-2 .. h0+258
        img_p = np.ascontiguousarray(shard.transpose(0, 2, 1)).astype(np.float16)

        # filts -> int8 [p, ht, di, plane, w]; plane order
        # [dj0 c0-2][dj2][dj1][dj3][dj4], odd dj (1,3) shifted +1 in w
        f = filts[b, h0 : h0 + HSH].reshape(N_HT, 128, W, K, K, C)
        q = np.clip(np.round(f / QSCALE), -127, 127).astype(np.int8)
        q = q.transpose(1, 0, 3, 4, 5, 2)  # [p, ht, di, dj, c, w]
        qr = np.empty((128, N_HT, K, K, C, W), dtype=np.int8)
        qr[:, :, :, 0] = q[:, :, :, 0]
        qr[:, :, :, 1] = q[:, :, :, 2]
        qr[:, :, :, 4] = q[:, :, :, 4]
        qr[:, :, :, 2:4, :, : W - 1] = q[:, :, :, 1::2, :, 1:]  # dj 1,3 shifted
        qr[:, :, :, 2:4, :, W - 1] = 0
        filts_p = np.ascontiguousarray(qr.reshape(128, N_HT, K, NPL, W))

        # exact fp16 edge filts: w=0, dj=3 -> [p, ht, di, c]
        edge_p = np.ascontiguousarray(
            f[:, :, 0, :, 3, :].transpose(1, 0, 2, 3)
        ).astype(np.float16)

        in_maps.append(
            {"img": img_p, "filts": filts_p, "edge": edge_p, "ident": ident}
        )
    return in_maps


def assemble_out(results) -> np.ndarray:
    out = np.empty((B, H, W), dtype=np.float32)
    for core in range(N_CORES):
        b, hh = divmod(core, 2)
        out[b, hh * HSH : (hh + 1) * HSH, :] = (
            results[core]["out"].reshape(HSH, W).astype(np.float32)
        )
    return out


def kernel(img_stack: np.ndarray, filts: np.ndarray) -> np.ndarray:
    nc = get_nc()
    in_maps = prepare_in_maps(img_stack, filts)
    res = run_bass_kernel_spmd(nc, in_maps, list(range(N_CORES)))
    return assemble_out(res.results)


# revision 19
# speedup vs baseline: 1.0895x; 1.0895x over previous
"""Per-pixel dynamic-filter 5x5 convolution (KPN-style) on 8 TRN2 NeuronCores.

Math: out[b,h,w] = sum_{di,dj,c} img[b, h+di-2, w+dj-2, c] * filts[b, h, w, (di*5+dj)*3+c]
Shapes: img [4,512,512,3] f32, filts [4,512,512,75] f32 -> out [4,512,512] f32.

Strategy (pure data parallel, no cross-core comms):
  - 8 shards = (batch b) x (H half); each core owns a [256, 512] output slab.
    Both 128-row h-tiles are fused into each op along a merged (c, ht) plane
    axis (uniform stride), so the kernel is just 5 di-iterations of large
    ops — few instructions, little semaphore traffic.
  - filts int8-quantized on host (scale S, ~9e-3 rel err vs 2e-2 budget),
    halving the dominant DMA stream. The 30 (dj, c, ht) planes per di split:
      dj in {0,2} (12 planes): ACT dequant -> DVE fp16 2x TT
      dj = 1      ( 6 planes): ACT dequant -> DVE fp16 2x TT
      dj = 3      ( 6 planes): DVE int8 x fp16 TT (1x), unscaled products
      dj = 4      ( 6 planes): GPSIMD int8 x fp16 TT, unscaled products
    Unscaled planes fold the scale into an S*I stationary at the PE step.
  - img fp16; one replicated-row DMA per di carries both h-tiles' rows
    (tile[p, c, t, x] = row p + di + 128 t) so engines never shift partitions.
  - odd dj (1,3) operands stay 4B-aligned by host-shifting those filts +1 in
    w (img x-offset dj+1, psum target cols [1:512)); the one real missing
    w=0 term (dj=3) is restored by a tiny TT chain merged at eviction.
  - TensorE accumulates planes into one fp32 PSUM bank per h-tile via I or
    S*I matmuls; ACT evicts cols [1:512) to fp16, DVE merges col 0 + edge.
  - Dummy-matmul warmup in the first DMA shadow lifts the PE HAM throttle.
"""

import sys

sys.path.insert(0, "/opt/trn_rl_repo")

import numpy as np

from concourse import bass, bacc, mybir
from concourse.tile import TileContext
from concourse.bass_utils import run_bass_kernel_spmd

B, H, W, C = 4, 512, 512, 3
K = 5
N_CORES = 8
HSH = H // 2  # 256 rows per shard
T = HSH // 128  # 2 h-tiles, fused on the (c,t) axis
XP = W + 6  # padded x extent: x = w+2, w in [-2, 516)
CT = C * T  # 6 merged (c,t) planes per dj
NG = K * CT  # 30 planes per di
QSCALE = float(np.float16(4.5 / 127.0))  # int8 quant scale, fp16-exact
N_WARMUP_MM = 14

_F16 = mybir.dt.float16
_F32 = mybir.dt.float32
_I8 = mybir.dt.int8

_NC = None

# plane-group order along the filts axis (per di, 6 (c,t) planes each):
#   [dj0][dj2][dj1][dj3][dj4]   -> indices 0:6, 6:12, 12:18, 18:24, 24:30
# within a group, planes are ordered (2c + t): c outer, h-tile t inner.


def build_nc():
    """Build the single-core Bass program (identical on all 8 cores)."""
    nc = bacc.Bacc("TRN2")
    img_d = nc.declare_dram_parameter("img", [HSH + 4, C, XP], _F16, isOutput=False)
    filts_d = nc.declare_dram_parameter("filts", [128, K, NG, W], _I8, isOutput=False)
    edge_d = nc.declare_dram_parameter("edge", [128, K, CT], _F16, isOutput=False)
    ident_d = nc.declare_dram_parameter("ident", [128, 128], _F16, isOutput=False)
    out_d = nc.declare_dram_parameter("out", [T, 128, W], _F16, isOutput=True)

    with TileContext(nc) as tc:
        with (
            tc.tile_pool(name="const", bufs=1) as constp,
            tc.tile_pool(name="imgp", bufs=3) as imgp,
            tc.tile_pool(name="qp", bufs=3) as qp,
            tc.tile_pool(name="fp", bufs=2) as fp,
            tc.tile_pool(name="prodp", bufs=2) as prodp,
            tc.tile_pool(name="accp", bufs=2) as accp,
            tc.tile_pool(name="outp", bufs=1) as outp,
            tc.tile_pool(name="psump", bufs=2, space="PSUM") as psump,
            tc.tile_pool(name="wpsump", bufs=1, space="PSUM") as wpsump,
        ):
            id_t = constp.tile([128, 128], _F16)
            nc.sync.dma_start(out=id_t[:], in_=ident_d[:])
            ids_t = constp.tile([128, 128], _F16, tag="ids")
            nc.scalar.mul(ids_t[:], id_t[:], QSCALE)  # S*I for unscaled planes
            edge_t = constp.tile([128, K, CT], _F16, tag="edge")
            nc.sync.dma_start(out=edge_t[:], in_=edge_d[:])

            # PE warmup: dummy matmuls in the first DMAs' shadow lift HAM.
            wsrc = constp.tile([128, 512], _F16, tag="wsrc")
            nc.vector.memset(wsrc[:], 0.0)
            wps = wpsump.tile([128, 512], _F32)
            for _ in range(N_WARMUP_MM):
                nc.tensor.matmul(wps[:], wsrc[:, :128], wsrc[:], start=True, stop=True)

            ps = [
                psump.tile([128, W], _F32, tag=f"ps{t}", name=f"ps{t}")
                for t in range(T)
            ]
            acc_prev = None

            for di in range(K):
                # img: tile[p, c, t, x] = img_d[p + di + 128 t, c, x]
                # (c,t) then has uniform plane stride XP in the merged view.
                img_t = imgp.tile([128, C, T, XP], _F16, tag="img", name=f"img{di}")
                src = img_d[di : di + 128, :, :]
                src = src.unsqueeze(2).broadcast_to((128, C, T, XP)).copy()
                src.ap[2] = [128 * C * XP, T]  # t steps 128 rows
                nc.sync.dma_start(out=img_t[:], in_=src)

                q_t = qp.tile([128, NG, W], _I8, tag="q", name=f"q{di}")
                nc.sync.dma_start(out=q_t[:, :18], in_=filts_d[:, di, :18])
                nc.sync.dma_start(out=q_t[:, 18:], in_=filts_d[:, di, 18:])

                # ACT dequants dj0, dj2, dj1 (18 planes)
                f_t = fp.tile([128, 18, W], _F16, tag="f", name=f"f{di}")
                nc.scalar.mul(f_t[:, :12], q_t[:, :12], QSCALE)
                nc.scalar.mul(f_t[:, 12:], q_t[:, 12:18], QSCALE)

                ctv = img_t[:].rearrange("p c t x -> p (c t) x")  # [p, 6, XP]

                # DVE TT-A: dj in {0,2}, x0 {0,2} (12 planes, fp16 2x)
                p_a = prodp.tile([128, 12, W], _F16, tag="pa", name=f"pa{di}")
                src_a = ctv.unsqueeze(1).broadcast_to((128, 2, CT, XP)).copy()
                src_a.ap[1] = [2, 2]  # dj axis: x offsets 0, 2
                src_a.ap[3] = [1, W]
                nc.vector.tensor_tensor(
                    p_a[:].rearrange("p (a g) x -> p a g x", a=2),
                    src_a,
                    f_t[:, :12].rearrange("p (a g) x -> p a g x", a=2),
                    mybir.AluOpType.mult,
                )
                # DVE TT-B: dj=1, x0=2 (6 planes, fp16 2x)
                p_b = prodp.tile([128, CT, W], _F16, tag="pb", name=f"pb{di}")
                nc.vector.tensor_tensor(
                    p_b[:], ctv[:, :, 2 : 2 + W], f_t[:, 12:18], mybir.AluOpType.mult
                )
                # DVE TT-C: dj=3, x0=4 (6 planes, int8 1x, unscaled)
                p_c = prodp.tile([128, CT, W], _F16, tag="pc", name=f"pc{di}")
                nc.vector.tensor_tensor(
                    p_c[:], q_t[:, 18:24], ctv[:, :, 4 : 4 + W], mybir.AluOpType.mult
                )
                # GPSIMD TT-D: dj=4, x0=4 (6 planes, int8, unscaled)
                p_d = prodp.tile([128, CT, W], _F16, tag="pd", name=f"pd{di}")
                nc.gpsimd.tensor_tensor(
                    p_d[:], q_t[:, 24:30], ctv[:, :, 4 : 4 + W], mybir.AluOpType.mult
                )

                # w=0 edge terms (dj=3): img x=3 dot exact-fp16 edge filts,
                # accumulated across di in fp32
                ep = accp.tile([128, CT], _F32, tag=f"ep{di % 2}", name=f"ep{di}")
                nc.vector.tensor_tensor(
                    ep[:], ctv[:, :, 3], edge_t[:, di], mybir.AluOpType.mult
                )
                if acc_prev is not None:
                    ac = accp.tile([128, CT], _F32, tag=f"ac{di % 2}", name=f"ac{di}")
                    nc.vector.tensor_tensor(
                        ac[:], ep[:], acc_prev[:], mybir.AluOpType.add
                    )
                    acc_prev = ac
                else:
                    acc_prev = ep

                # accumulate the 30 planes; bank = plane's t (index % 2)
                def mm(tile, j, stat, odd, start=False, stop=False):
                    bank = ps[j % 2]
                    if odd:
                        nc.tensor.matmul(
                            bank[:, 1:W], stat, tile[:, j, 0 : W - 1],
                            start=False, stop=stop,
                        )
                    else:
                        nc.tensor.matmul(
                            bank[:], stat, tile[:, j, :], start=start, stop=stop,
                        )

                for j in range(12):
                    mm(p_a, j, id_t[:], odd=False, start=(di == 0 and j < 2))
                for j in range(CT):
                    mm(p_d, j, ids_t[:], odd=False)
                for j in range(CT):
                    mm(p_b, j, id_t[:], odd=True)
                for j in range(CT):
                    mm(p_c, j, ids_t[:], odd=True,
                       stop=(di == K - 1 and j >= CT - 2))

            # edge reduce: (2c+t) -> per-t sums [e_t0, e_t1]
            s1 = accp.tile([128, 2], _F32, tag="s1")
            nc.vector.tensor_tensor(
                s1[:], acc_prev[:, 0:2], acc_prev[:, 2:4], mybir.AluOpType.add
            )
            s2 = accp.tile([128, 2], _F32, tag="s2")
            nc.vector.tensor_tensor(
                s2[:], s1[:], acc_prev[:, 4:6], mybir.AluOpType.add
            )

            # evict: ACT copies cols [1:512); DVE merges edge into col 0
            for t in range(T):
                o_t = outp.tile([128, W], _F16, tag=f"ot{t}", name=f"ot{t}")
                nc.scalar.copy(out=o_t[:, 1:], in_=ps[t][:, 1:])
                nc.vector.tensor_tensor(
                    o_t[:, 0:1], ps[t][:, 0:1], s2[:, t : t + 1],
                    mybir.AluOpType.add,
                )
                nc.sync.dma_start(out=out_d[t], in_=o_t[:])

    nc.compile()
    return nc


def get_nc():
    global _NC
    if _NC is None:
        _NC = build_nc()
    return _NC


def prepare_in_maps(img_stack: np.ndarray, filts: np.ndarray):
    """Shard + reformat FULL fp32 inputs into per-core input maps."""
    ident = np.eye(128, dtype=np.float16)
    in_maps = []
    for core in range(N_CORES):
        b, hh = divmod(core, 2)
        h0 = hh * HSH
        # img: pad h by 2 each side, w by 2 left / 4 right -> [516, 518, 3]
        padded = np.pad(img_stack[b], ((2, 2), (2, XP - W - 2), (0, 0)))
        shard = padded[h0 : h0 + HSH + 4]  # rows h0-2 .. h0+258
        img_p = np.ascontiguousarray(shard.transpose(0, 2, 1)).astype(np.float16)

        # filts -> int8 [p, di, group, w]; group = [dj0][dj2][dj1][dj3][dj4]
        # x (2c+t); odd dj (1,3) pre-shifted +1 in w
        f = filts[b, h0 : h0 + HSH].reshape(T, 128, W, K, K, C)
        q = np.clip(np.round(f / QSCALE), -127, 127).astype(np.int8)
        # -> [p, di, dj, c, t, w]
        q = q.transpose(1, 3, 4, 5, 0, 2)
        qr = q[:, :, [0, 2, 1, 3, 4]].copy()  # dj order
        qr[:, :, 2:4, :, :, : W - 1] = qr[:, :, 2:4, :, :, 1:].copy()  # dj 1,3 +1
        qr[:, :, 2:4, :, :, W - 1] = 0
        filts_p = np.ascontiguousarray(qr.reshape(128, K, NG, W))

        # exact fp16 edge filts (w=0, dj=3) -> [p, di, (2c+t)]
        e = f[:, :, 0, :, 3, :]  # [t, p, di, c]
        edge_p = np.ascontiguousarray(e.transpose(1, 2, 3, 0).reshape(128, K, CT)).astype(
            np.float16
        )

        in_maps.append(
            {"img": img_p, "filts": filts_p, "edge": edge_p, "ident": ident}
        )
    return in_maps


def assemble_out(results) -> np.ndarray:
    out = np.empty((B, H, W), dtype=np.float32)
    for core in range(N_CORES):
        b, hh = divmod(core, 2)
        out[b, hh * HSH : (hh + 1) * HSH, :] = (
            results[core]["out"].reshape(HSH, W).astype(np.float32)
        )
    return out


def kernel(img_stack: np.ndarray, filts: np.ndarray) -> np.ndarray:
    nc = get_nc()
    in_maps = prepare_in_maps(img_stack, filts)
    res = run_bass_kernel_spmd(nc, in_maps, list(range(N_CORES)))
    return assemble_out(res.results)


# revision 20
# speedup vs baseline: 1.1671x; 1.0712x over previous
"""Per-pixel dynamic-filter 5x5 convolution (KPN-style) on 8 TRN2 NeuronCores.

Math: out[b,h,w] = sum_{di,dj,c} img[b, h+di-2, w+dj-2, c] * filts[b, h, w, (di*5+dj)*3+c]
Shapes: img [4,512,512,3] f32, filts [4,512,512,75] f32 -> out [4,512,512] f32.

Strategy (pure data parallel, no cross-core comms):
  - 8 shards = (batch b in 0..3) x (H half in 0..1); each core owns a
    [256, 512] output slab (2 fused 128-row h-tiles).
  - Host prep (per core): img padded + transposed to [h', c, x] fp16; filts
    transposed to [p, di, dj, c, ht, w] fp16 so each (di,dj) group of six
    (c,ht) planes sits at uniform stride.
  - On-chip per di: one img tile [p][c][ht][x520] (rows DMA'd at offset di)
    plus a one-element-x-shifted copy (ACT) so odd-dj operands stay
    4B-aligned. One DVE tensor_tensor per (di,dj) computes all six (c,ht)
    product planes in a single FD=3072 fp16 2x-mode instruction (25 TTs
    total). The TensorEngine accumulates the planes into two fp32 PSUM
    banks via identity matmuls; ACT evicts, DMA out.
  - Dummy-matmul warmup inside the first DMA shadow lifts the PE HAM
    clock throttle; per-di chunking puts the first TT ~6us in.
"""

import sys

sys.path.insert(0, "/opt/trn_rl_repo")

import numpy as np

from concourse import bass, bacc, mybir
from concourse.tile import TileContext
from concourse.bass_utils import run_bass_kernel_spmd

B, H, W, C = 4, 512, 512, 3
K = 5
KK = K * K * C  # 75
N_CORES = 8
HSH = H // 2  # 256 rows per shard
XP = W + 6  # img DRAM x extent: w in [-2, 516) -> x = w+2 in [0, 518)
XT = XP  # x extent in SBUF img tiles (even, so the (c,ht) plane stride stays 4B-aligned)
IMG_FREE = C * XP  # 1554 per padded DRAM img row
N_HT = HSH // 128  # 2 h-tiles per shard, fused
N_WARMUP_MM = 10

_F16 = mybir.dt.float16
_F32 = mybir.dt.float32

_NC = None


def build_nc():
    """Build the single-core Bass program (identical on all 8 cores)."""
    nc = bacc.Bacc("TRN2")
    NP = C * N_HT  # (c,ht) planes per (di,dj)
    img_d = nc.declare_dram_parameter("img", [HSH + 4, C, XP], _F16, isOutput=False)
    filts_d = nc.declare_dram_parameter(
        "filts", [128, K, K, C, N_HT, W], _F16, isOutput=False
    )
    ident_d = nc.declare_dram_parameter("ident", [128, 128], _F16, isOutput=False)
    out_d = nc.declare_dram_parameter("out", [HSH, W], _F32, isOutput=True)

    with TileContext(nc) as tc:
        with (
            tc.tile_pool(name="const", bufs=1) as constp,
            tc.tile_pool(name="imgp", bufs=3) as imgp,
            tc.tile_pool(name="filtp", bufs=3) as filtp,
            tc.tile_pool(name="prodp", bufs=3) as prodp,
            tc.tile_pool(name="outp", bufs=2) as outp,
            tc.tile_pool(name="psump", bufs=1, space="PSUM") as psump,
            tc.tile_pool(name="wpsump", bufs=1, space="PSUM") as wpsump,
        ):
            id_t = constp.tile([128, 128], _F16)
            nc.sync.dma_start(out=id_t[:], in_=ident_d[:])

            # PE warmup: dummy matmuls in the first DMAs' shadow lift HAM.
            wsrc = constp.tile([128, 512], _F16, tag="wsrc")
            nc.gpsimd.memset(wsrc[:], 0.0)
            wps = wpsump.tile([128, 512], _F32)
            for _ in range(N_WARMUP_MM):
                nc.tensor.matmul(wps[:], wsrc[:, :128], wsrc[:], start=True, stop=True)

            psum_t = [
                psump.tile([128, W], _F32, tag=f"ps{ht}", name=f"ps{ht}")
                for ht in range(N_HT)
            ]

            for di in range(K):
                # img tile layout [p][c][ht][x:XT]; plane k = N_HT*c + ht at
                # uniform stride XT. Rows at partition offset di.
                t0 = imgp.tile([128, C, N_HT, XT], _F16, tag="img0", name=f"img0_{di}")
                for ht in range(N_HT):
                    nc.sync.dma_start(
                        out=t0[:, :, ht, :XP],
                        in_=img_d[ht * 128 + di : ht * 128 + di + 128, :, :],
                    )
                # x-shifted-by-one copy keeps odd-dj operands 4B-aligned
                t1 = imgp.tile([128, C, N_HT, XT], _F16, tag="img1", name=f"img1_{di}")
                fl0 = t0[:].rearrange("p c t x -> p (c t x)")
                fl1 = t1[:].rearrange("p c t x -> p (c t x)")
                nfree = C * N_HT * XT
                nc.scalar.copy(out=fl1[:, 0 : nfree - 1], in_=fl0[:, 1:nfree])
                imgs = {0: t0, 1: t1}

                # filts for this di: [p][dj][c][ht][w], contiguous per
                # partition; split into two sub-DMAs for earlier first-use.
                ft = filtp.tile([128, K, C, N_HT, W], _F16, tag="ft", name=f"ft{di}")
                nc.sync.dma_start(out=ft[:, :2], in_=filts_d[:, di, :2])
                nc.sync.dma_start(out=ft[:, 2:], in_=filts_d[:, di, 2:])

                for dj in range(K):
                    q = dj & 1
                    x0 = dj - q
                    p_t = prodp.tile([128, NP, W], _F16, tag="pt", name=f"pt{di}{dj}")
                    src = imgs[q][:].rearrange("p c t x -> p (c t) x")
                    nc.vector.tensor_tensor(
                        p_t[:],
                        src[:, :, x0 : x0 + W],
                        ft[:, dj].rearrange("p c t w -> p (c t) w"),
                        mybir.AluOpType.mult,
                    )
                    first = di == 0 and dj == 0
                    last = di == K - 1 and dj == K - 1
                    for k in range(NP):
                        ht = k % N_HT
                        nc.tensor.matmul(
                            psum_t[ht][:],
                            id_t[:],
                            p_t[:, k, :],
                            start=(first and k < N_HT),
                            stop=(last and k >= NP - N_HT),
                        )

            for ht in range(N_HT):
                o_t = outp.tile([128, W], _F32, tag="ot", name=f"ot{ht}")
                nc.scalar.copy(out=o_t[:], in_=psum_t[ht][:])
                nc.sync.dma_start(out=out_d[ht * 128 : (ht + 1) * 128, :], in_=o_t[:])

    nc.compile()
    return nc


def get_nc():
    global _NC
    if _NC is None:
        _NC = build_nc()
    return _NC


def prepare_in_maps(img_stack: np.ndarray, filts: np.ndarray):
    """Shard + reformat FULL fp32 inputs into per-core fp16 input maps."""
    ident = np.eye(128, dtype=np.float16)
    in_maps = []
    for core in range(N_CORES):
        b, hh = divmod(core, 2)
        h0 = hh * HSH
        # img: pad h by 2 each side, w by 2 left / 4 right -> [516, 518, 3]
        padded = np.pad(img_stack[b], ((2, 2), (2, XP - W - 2), (0, 0)))
        shard = padded[h0 : h0 + HSH + 4]  # rows h0-2 .. h0+258
        img_p = np.ascontiguousarray(shard.transpose(0, 2, 1)).astype(np.float16)
        # filts -> [p, di, dj, c, ht, w]
        f = filts[b, h0 : h0 + HSH].reshape(N_HT, 128, W, K, K, C)
        filts_p = np.ascontiguousarray(f.transpose(1, 3, 4, 5, 0, 2)).astype(
            np.float16
        )
        in_maps.append({"img": img_p, "filts": filts_p, "ident": ident})
    return in_maps


def assemble_out(results) -> np.ndarray:
    out = np.empty((B, H, W), dtype=np.float32)
    for core in range(N_CORES):
        b, hh = divmod(core, 2)
        out[b, hh * HSH : (hh + 1) * HSH, :] = results[core]["out"]
    return out


def kernel(img_stack: np.ndarray, filts: np.ndarray) -> np.ndarray:
    nc = get_nc()
    in_maps = prepare_in_maps(img_stack, filts)
    res = run_bass_kernel_spmd(nc, in_maps, list(range(N_CORES)))
    return assemble_out(res.results)
